# revision 1
# baseline (speedup 1.0000x reference)
"""D-MPNN layer on 8 TRN2 NeuronCores (Bass/Tile, SPMD).

out = (1-z)*s + z*m with
  mess_ki = mess[nei_idx]                       [M, D]
  s_ij    = segment_sum(mess_ki, src_idx, E)    [E, D]
  z_ij    = sigmoid([h_ij | s_ij] @ Wz + bz)    [E, D]
  r_ki    = sigmoid([h_ki | mess_ki] @ Wr + br) [M, D]
  r_ij    = segment_sum(r_ki*mess_ki, src, E)   [E, D]
  m_ij    = tanh(h_ij @ W + bw + r_ij @ U)      [E, D]

Sharding: edges E split into 8 contiguous chunks (EC=E/8); each M-row is
routed on host to the core owning its src edge, so segment sums are
core-local (no collectives).  Within a core, rows (sorted by src) are
greedily packed into variable-width dst blocks: each block covers a run of
consecutive dst edges (window <= 128 wide) holding <= 384 rows (3 row
tiles), padded to exactly 384.  A final 4-tile block covers the core's last
128 dst edges.  All cores share one static program (block count padded to a
common B2 with dummy blocks); per-block dst bases live only in host-side
data (h_ij chunks, srcrel) and in the host-side un-permutation of the
block-indexed device output.

Per 128-row tile the device computes r = sigmoid(X @ Wr) (X^T streamed
pre-transposed; gathered messages pre-gathered on host), then aggregates
s^T/r^T (and row-major s) per block with one-hot matmuls
(onehot[m,d] = [srcrel[m]==d] from an iota/is_equal compare).  Per block:
z/m matmuls consume h_ij^T and the transposed s/r straight from PSUM-copied
SBUF, sigmoid/tanh, combine with row-major s, DMA out.
"""

import numpy as np
import ml_dtypes

BF16 = ml_dtypes.bfloat16

E = 262144
M = 786432
F_NB = 192
D = 256
NCORES = 8

FULL_DIMS = dict(E=E, M=M, F=F_NB, D=D, ncores=NCORES, BLK=128, C=384,
                 CT=512, KG=6)


def _dims(d, B2):
    o = dict(d)
    o["B2"] = B2
    o["EC"] = o["E"] // o["ncores"]
    o["TPB"] = o["C"] // 128              # 3 row tiles per normal block
    o["TPT"] = o["CT"] // 128             # 4 row tiles in the tail block
    assert o["KG"] == 2 * o["TPB"]
    o["BPG"] = 2
    assert B2 % 2 == 0
    o["G"] = B2 // 2
    o["T"] = o["TPB"] * B2 + o["TPT"]     # total row tiles per core
    return o


def _greedy_blocks(csum, EC, C):
    """Greedy variable-width blocks over dst edges [0, EC-128).
    csum[i] = rows with dst < i.  Returns block base list."""
    bases = []
    i = 0
    while i < EC - 128:
        base = i
        hi = min(base + 128, EC - 128)
        j = int(np.searchsorted(csum, csum[base] + C, side="right")) - 1
        j = max(base + 1, min(j, hi))
        bases.append(base)
        i = j
    return bases


def host_prep(inputs, dims=FULL_DIMS):
    dm0 = dict(dims)
    EC = dm0["E"] // dm0["ncores"]
    C, CT, KG = dm0["C"], dm0["CT"], dm0["KG"]
    F, Dd = dm0["F"], dm0["D"]
    ncores = dm0["ncores"]
    TPB = C // 128

    src = np.asarray(inputs["src_idx"]).astype(np.int64).ravel()
    nei = np.asarray(inputs["nei_idx"]).astype(np.int64).ravel()
    h_ij = np.asarray(inputs["h_ij"])
    h_ki = np.asarray(inputs["h_ki"])
    mess = np.asarray(inputs["mess"])

    order = np.argsort(src, kind="stable")
    src_s = src[order]
    nei_s = nei[order]
    cnt = np.bincount(src_s, minlength=dm0["E"])

    core_blocks = []
    for c in range(ncores):
        csum = np.concatenate(
            [[0], np.cumsum(cnt[c * EC:(c + 1) * EC])]
        )
        bases = _greedy_blocks(csum, EC, C)
        tail_rows = csum[EC] - csum[EC - 128]
        if tail_rows > CT:
            raise OverflowError(f"tail rows {tail_rows} > CT={CT}")
        core_blocks.append((bases, csum))
    nreal = [len(b[0]) for b in core_blocks]
    B2 = max(nreal)
    B2 += B2 % 2
    dm = _dims(dm0, B2)
    G, T = dm["G"], dm["T"]
    TPT = dm["TPT"]

    mess_bf = mess.astype(BF16)
    h_ki_bf = h_ki[order].astype(BF16)
    mess_g_all = mess_bf[nei_s]            # [M, D] gathered, src-sorted
    wz = np.ascontiguousarray(np.asarray(inputs["Wz_w"]).astype(BF16))
    wr = np.ascontiguousarray(np.asarray(inputs["Wr_w"]).astype(BF16))
    u = np.ascontiguousarray(np.asarray(inputs["U_w"]).astype(BF16))
    w = np.ascontiguousarray(np.asarray(inputs["W_w"]).astype(BF16))

    row_lo = np.searchsorted(src_s, np.arange(ncores) * EC)
    row_hi = np.searchsorted(src_s, (np.arange(ncores) + 1) * EC)

    in_maps = []
    metas = []
    for c in range(ncores):
        bases, csum = core_blocks[c]
        nb = len(bases)
        ndummy = B2 - nb
        MPC = B2 * C + CT
        rlo = row_lo[c]
        nrow_core = row_hi[c] - rlo

        bases_arr = np.asarray(bases, dtype=np.int64)
        nexts = np.concatenate([bases_arr[1:], [EC - 128]])
        widths = nexts - bases_arr
        rs = csum[bases_arr]               # first row of each block
        tail_start = csum[EC - 128]

        # per-row block id (for rows before the tail)
        rowblk = np.zeros(nrow_core, np.int64)
        rowblk[rs[1:][rs[1:] < nrow_core]] += 1
        rowblk = np.cumsum(rowblk)
        blk_of_row = np.minimum(rowblk, nb - 1)
        ridx = np.arange(nrow_core)
        is_tail = ridx >= tail_start
        pos_in_blk = ridx - rs[blk_of_row]
        slot_of_row = np.where(
            is_tail,
            B2 * C + (ridx - tail_start),
            (ndummy + blk_of_row) * C + pos_in_blk,
        )
        base_of_row = np.where(is_tail, EC - 128, bases_arr[blk_of_row])
        srcrel_pad = np.full(MPC, 999.0, np.float32)
        srcrel_pad[slot_of_row] = (
            src_s[rlo:row_hi[c]] - c * EC - base_of_row
        ).astype(np.float32)

        h_pad = np.zeros((MPC, F), BF16)
        h_pad[slot_of_row] = h_ki_bf[rlo:row_hi[c]]
        mg_pad = np.zeros((MPC, Dd), BF16)
        mg_pad[slot_of_row] = mess_g_all[rlo:row_hi[c]]

        # h_ij chunks per block (dummies zero), [B2+1, 128, F]
        hij_all = np.zeros((B2 + 1, 128, F), BF16)
        hijc = h_ij[c * EC:(c + 1) * EC].astype(BF16)
        gather_rows = bases_arr[:, None] + np.arange(128)[None, :]
        hij_all[ndummy:B2] = hijc[gather_rows]
        hij_all[B2] = hijc[EC - 128:]

        # ---- tile layouts ----
        NT = B2 * TPB
        src_all = np.ascontiguousarray(srcrel_pad.reshape(T, 128).T)

        def tileify(arr2d, ntiles, off_rows):
            a = arr2d[off_rows:off_rows + ntiles * 128]
            return a.reshape(ntiles, 128, -1).transpose(0, 2, 1)

        mgn = mg_pad[:NT * 128].reshape(G, KG, 128, Dd)
        mg_l = mgn.transpose(0, 2, 1, 3).reshape(G, 128, KG * Dd)
        mt3 = tileify(mg_pad, NT, 0)
        mta = (mt3[:, :128, :].reshape(G, KG, 128, 128)
               .transpose(0, 2, 1, 3).reshape(G, 128, KG * 128))
        mtb = (mt3[:, 128:, :].reshape(G, KG, 128, 128)
               .transpose(0, 2, 1, 3).reshape(G, 128, KG * 128))
        h3 = tileify(h_pad, NT, 0)
        ha = (h3[:, :128, :].reshape(G, KG, 128, 128)
              .transpose(0, 2, 1, 3).reshape(G, 128, KG * 128))
        hb = (h3[:, 128:F, :].reshape(G, KG // 2, 2, 64, 128)
              .transpose(0, 2, 3, 1, 4).reshape(G, 128, (KG // 2) * 128))
        hijt = hij_all[:B2].transpose(0, 2, 1)
        hija = (hijt[:, :128, :].reshape(G, 2, 128, 128)
                .transpose(0, 2, 1, 3).reshape(G, 128, 2 * 128))
        hijb = (hijt[:, 128:F, :].reshape(G, 1, 2, 64, 128)
                .transpose(0, 2, 3, 1, 4).reshape(G, 128, 128))
        blob = np.ascontiguousarray(
            np.concatenate([mg_l, mta, mtb, ha, hb, hija, hijb], axis=2)
        )

        # tail section (4 tiles, one block)
        toff = NT * 128
        mgt = (mg_pad[toff:].reshape(TPT, 128, Dd)
               .transpose(1, 0, 2).reshape(128, TPT * Dd))
        mtt = tileify(mg_pad, TPT, toff)
        mtta = mtt[:, :128, :].transpose(1, 0, 2).reshape(128, TPT * 128)
        mttb = mtt[:, 128:, :].transpose(1, 0, 2).reshape(128, TPT * 128)
        ht3 = tileify(h_pad, TPT, toff)
        hta = ht3[:, :128, :].transpose(1, 0, 2).reshape(128, TPT * 128)
        htb = (ht3[:, 128:F, :].reshape(2, 2, 64, 128)
               .transpose(1, 2, 0, 3).reshape(128, 2 * 128))
        htij = hij_all[B2].T
        tail = np.ascontiguousarray(
            np.concatenate([mgt, mtta, mttb, hta, htb, htij[:128]], axis=1)
        )
        htijb = np.ascontiguousarray(htij[128:F])

        in_maps.append(
            dict(srcrel=src_all, blob=blob, tail=tail, htijb=htijb,
                 wz=wz, wr=wr, u=u, w=w)
        )
        metas.append(dict(bases=bases_arr, widths=widths, ndummy=ndummy))
    return in_maps, metas, dm


def build_program(dm):
    import concourse.tile as tile
    from concourse import bacc, mybir

    EC, KG, T, G, B2 = dm["EC"], dm["KG"], dm["T"], dm["G"], dm["B2"]
    TPB, TPT, F, Dd = dm["TPB"], dm["TPT"], dm["F"], dm["D"]
    f32 = mybir.dt.float32
    bf16 = mybir.dt.bfloat16
    i32 = mybir.dt.int32
    AF = mybir.ActivationFunctionType
    ALU = mybir.AluOpType

    nc = bacc.Bacc("TRN2", target_bir_lowering=False, debug=False,
                   num_devices=dm["ncores"])

    SEG = [KG * Dd, KG * 128, KG * 128, KG * 128, (KG // 2) * 128,
           2 * 128, 128]
    SEGOFF = [0]
    for sgl in SEG:
        SEGOFF.append(SEGOFF[-1] + sgl)
    SEGT = [TPT * Dd, TPT * 128, TPT * 128, TPT * 128, 2 * 128, 128]
    SEGTOFF = [0]
    for sgl in SEGT:
        SEGTOFF.append(SEGTOFF[-1] + sgl)

    srcrel_d = nc.dram_tensor("srcrel", [128, T], f32, kind="ExternalInput")
    blob_d = nc.dram_tensor("blob", [G, 128, SEGOFF[-1]], bf16,
                            kind="ExternalInput")
    tail_d = nc.dram_tensor("tail", [128, SEGTOFF[-1]], bf16,
                            kind="ExternalInput")
    htijb_d = nc.dram_tensor("htijb", [64, 128], bf16, kind="ExternalInput")
    wz_d = nc.dram_tensor("wz", [F + Dd, Dd], bf16, kind="ExternalInput")
    wr_d = nc.dram_tensor("wr", [F + Dd, Dd], bf16, kind="ExternalInput")
    u_d = nc.dram_tensor("u", [Dd, Dd], bf16, kind="ExternalInput")
    w_d = nc.dram_tensor("w", [F, Dd], bf16, kind="ExternalInput")
    y_d = nc.dram_tensor("y", [(B2 + 1) * 128, Dd], f32,
                         kind="ExternalOutput")

    with tile.TileContext(nc) as tc:
        with (
            tc.tile_pool(name="const", bufs=1) as const,
            tc.tile_pool(name="gat", bufs=4) as gat,
            tc.tile_pool(name="mid", bufs=4) as mid,
            tc.tile_pool(name="fin", bufs=4) as fin,
            tc.tile_pool(name="psA", bufs=2, space="PSUM") as psA,
            tc.tile_pool(name="psS", bufs=4, space="PSUM") as psS,
            tc.tile_pool(name="psR", bufs=2, space="PSUM") as psR,
        ):
            iota_i = const.tile([128, 128], i32)
            nc.gpsimd.iota(iota_i[:], pattern=[[1, 128]], base=0,
                           channel_multiplier=0)
            iota_f = const.tile([128, 128], f32)
            nc.vector.tensor_copy(iota_f[:], iota_i[:])

            def load_w(dram, ks, nm):
                tiles = []
                r0 = 0
                for i, k in enumerate(ks):
                    t = const.tile([k, Dd], bf16, tag=f"{nm}{i}")
                    nc.sync.dma_start(out=t[:], in_=dram[r0:r0 + k, :])
                    tiles.append(t)
                    r0 += k
                return tiles

            wr_t = load_w(wr_d, (128, 64, 128, 128), "wr")
            wr1d = const.tile([128, Dd], bf16, tag="wr1d")
            nc.sync.dma_start(out=wr1d[0:64, :], in_=wr_d[128:192, :])
            nc.sync.dma_start(out=wr1d[64:128, :], in_=wr_d[128:192, :])
            wz_t = load_w(wz_d, (128, 64, 128, 128), "wz")
            w_t = load_w(w_d, (128, 64), "w")
            u_t = load_w(u_d, (128, 128), "u")
            zw0 = const.tile([128, 2 * Dd], bf16, tag="zw0")
            nc.sync.dma_start(out=zw0[:, 0:Dd], in_=wz_d[0:128, :])
            nc.sync.dma_start(out=zw0[:, Dd:2 * Dd], in_=w_d[0:128, :])
            zw1 = const.tile([128, 2 * Dd], bf16, tag="zw1")
            for half in (0, 64):
                nc.sync.dma_start(out=zw1[half:half + 64, 0:Dd],
                                  in_=wz_d[128:192, :])
                nc.sync.dma_start(out=zw1[half:half + 64, Dd:2 * Dd],
                                  in_=w_d[128:192, :])

            src_all = const.tile([128, T], f32)
            nc.sync.dma_start(out=src_all[:], in_=srcrel_d[:, :])

            def do_tiles(ntile, t0, mess_g, mta_sb, mtb_sb, ha_sb, hb_sb):
                oh_g = mid.tile([128, KG, 128], bf16, tag="oh")
                nc.vector.tensor_tensor(
                    out=oh_g[:, :ntile, :],
                    in0=src_all[:, t0:t0 + ntile, None].broadcast_to(
                        [128, ntile, 128]),
                    in1=iota_f[:, None, :].broadcast_to([128, ntile, 128]),
                    op=ALU.is_equal,
                )
                r_g = mid.tile([128, KG * Dd], bf16, tag="rg")
                for j in range(ntile):
                    pr = psS.tile([128, Dd], f32, tag="ps")
                    nc.tensor.matmul(out=pr[:],
                                     lhsT=ha_sb[:, j * 128:(j + 1) * 128],
                                     rhs=wr_t[0][:], start=True, stop=False)
                    half = (j % 2) * 64
                    hb_t = hb_sb[half:half + 64,
                                 (j // 2) * 128:(j // 2 + 1) * 128]
                    nc.tensor.matmul(out=pr[:], lhsT=hb_t,
                                     rhs=wr1d[half:half + 64, :],
                                     start=False, stop=False)
                    nc.tensor.matmul(out=pr[:],
                                     lhsT=mta_sb[:, j * 128:(j + 1) * 128],
                                     rhs=wr_t[2][:], start=False, stop=False)
                    nc.tensor.matmul(out=pr[:],
                                     lhsT=mtb_sb[:, j * 128:(j + 1) * 128],
                                     rhs=wr_t[3][:], start=False, stop=True)
                    nc.scalar.activation(r_g[:, j * Dd:(j + 1) * Dd], pr[:],
                                         AF.Sigmoid)
                rm_g = mid.tile([128, KG * Dd], bf16, tag="rm")
                nc.vector.tensor_tensor(out=rm_g[:, :ntile * Dd],
                                        in0=r_g[:, :ntile * Dd],
                                        in1=mess_g[:, :ntile * Dd],
                                        op=ALU.mult)
                return r_g, rm_g, oh_g

            def do_block(b, ntile, j0, oh_g, mess_g, rm_g,
                         hija_t, hijb_t, bhalf):
                pa = psA.tile([128, 4 * 128], f32, tag="pa")
                psr = psR.tile([128, Dd], f32, tag="psr")
                for tj in range(ntile):
                    j = j0 + tj
                    oh = oh_g[:, j, :]
                    mess_t = mess_g[:, j * Dd:(j + 1) * Dd]
                    rm_t = rm_g[:, j * Dd:(j + 1) * Dd]
                    st = tj == 0
                    sp = tj == ntile - 1
                    nc.tensor.matmul(out=pa[:, 0:128], lhsT=mess_t[:, 0:128],
                                     rhs=oh, start=st, stop=False)
                    nc.tensor.matmul(out=pa[:, 128:256],
                                     lhsT=mess_t[:, 128:256],
                                     rhs=oh, start=False, stop=False)
                    nc.tensor.matmul(out=pa[:, 256:384], lhsT=rm_t[:, 0:128],
                                     rhs=oh, start=False, stop=False)
                    nc.tensor.matmul(out=pa[:, 384:512],
                                     lhsT=rm_t[:, 128:256],
                                     rhs=oh, start=False, stop=sp)
                    nc.tensor.matmul(out=psr[:], lhsT=oh, rhs=mess_t,
                                     start=st, stop=sp)

                sr_sb = fin.tile([128, 512], bf16, tag="sr")
                nc.vector.tensor_copy(sr_sb[:, 0:256], pa[:, 0:256])
                nc.scalar.copy(sr_sb[:, 256:512], pa[:, 256:512])

                pzm = psS.tile([128, 512], f32, tag="ps")
                nc.tensor.matmul(out=pzm[:, 0:512], lhsT=hija_t,
                                 rhs=zw0[:], start=True, stop=False)
                nc.tensor.matmul(out=pzm[:, 0:512], lhsT=hijb_t,
                                 rhs=zw1[bhalf:bhalf + 64, :],
                                 start=False, stop=False)
                nc.tensor.matmul(out=pzm[:, 0:256], lhsT=sr_sb[:, 0:128],
                                 rhs=wz_t[2][:], start=False, stop=False)
                nc.tensor.matmul(out=pzm[:, 0:256], lhsT=sr_sb[:, 128:256],
                                 rhs=wz_t[3][:], start=False, stop=False)
                nc.tensor.matmul(out=pzm[:, 256:512], lhsT=sr_sb[:, 256:384],
                                 rhs=u_t[0][:], start=False, stop=False)
                nc.tensor.matmul(out=pzm[:, 256:512], lhsT=sr_sb[:, 384:512],
                                 rhs=u_t[1][:], start=False, stop=True)

                z_sb = fin.tile([128, Dd], f32, tag="z")
                nc.scalar.activation(z_sb[:], pzm[:, 0:256], AF.Sigmoid)
                m_sb = fin.tile([128, Dd], f32, tag="m")
                nc.scalar.activation(m_sb[:], pzm[:, 256:512], AF.Tanh)

                d_sb = fin.tile([128, Dd], f32, tag="d")
                nc.vector.tensor_tensor(out=d_sb[:], in0=m_sb[:], in1=psr[:],
                                        op=ALU.subtract)
                nc.vector.tensor_tensor(out=d_sb[:], in0=d_sb[:], in1=z_sb[:],
                                        op=ALU.mult)
                o_sb = fin.tile([128, Dd], f32, tag="o")
                nc.vector.tensor_tensor(out=o_sb[:], in0=d_sb[:], in1=psr[:],
                                        op=ALU.add)
                nc.scalar.dma_start(out=y_d[b * 128:(b + 1) * 128, :],
                                    in_=o_sb[:])

            for g in range(G):
                blob_sb = gat.tile([128, SEGOFF[-1]], bf16, tag="blob")
                nc.sync.dma_start(out=blob_sb[:], in_=blob_d[g])
                mess_g = blob_sb[:, SEGOFF[0]:SEGOFF[1]]
                mta_sb = blob_sb[:, SEGOFF[1]:SEGOFF[2]]
                mtb_sb = blob_sb[:, SEGOFF[2]:SEGOFF[3]]
                ha_sb = blob_sb[:, SEGOFF[3]:SEGOFF[4]]
                hb_sb = blob_sb[:, SEGOFF[4]:SEGOFF[5]]
                hija_sb = blob_sb[:, SEGOFF[5]:SEGOFF[6]]
                hijb_sb = blob_sb[:, SEGOFF[6]:SEGOFF[7]]

                r_g, rm_g, oh_g = do_tiles(KG, g * KG, mess_g, mta_sb,
                                           mtb_sb, ha_sb, hb_sb)
                for bb in range(2):
                    do_block(
                        g * 2 + bb, TPB, bb * TPB, oh_g, mess_g, rm_g,
                        hija_sb[:, bb * 128:(bb + 1) * 128],
                        hijb_sb[(bb % 2) * 64:(bb % 2) * 64 + 64, 0:128],
                        (bb % 2) * 64,
                    )

            # tail block
            tail_sb = gat.tile([128, SEGTOFF[-1]], bf16, tag="tail")
            nc.sync.dma_start(out=tail_sb[:], in_=tail_d[:, :])
            htijb_sb = gat.tile([64, 128], bf16, tag="htijb")
            nc.sync.dma_start(out=htijb_sb[:], in_=htijb_d[:, :])
            r_g, rm_g, oh_g = do_tiles(
                TPT, B2 * TPB,
                tail_sb[:, SEGTOFF[0]:SEGTOFF[1]],
                tail_sb[:, SEGTOFF[1]:SEGTOFF[2]],
                tail_sb[:, SEGTOFF[2]:SEGTOFF[3]],
                tail_sb[:, SEGTOFF[3]:SEGTOFF[4]],
                tail_sb[:, SEGTOFF[4]:SEGTOFF[5]],
            )
            do_block(B2, TPT, 0, oh_g,
                     tail_sb[:, SEGTOFF[0]:SEGTOFF[1]], rm_g,
                     tail_sb[:, SEGTOFF[5]:SEGTOFF[6]],
                     htijb_sb[0:64, 0:128], 0)

    nc.compile()
    return nc


_CACHE = {}
LAST_RESULT = None


def kernel(**inputs):
    from concourse.bass_utils import run_bass_kernel_spmd

    for b in ("Wz_b", "Wr_b", "W_b"):
        assert not np.any(np.asarray(inputs[b])), f"nonzero bias {b} unsupported"

    in_maps, metas, dm = host_prep(inputs, FULL_DIMS)
    key = (tuple(sorted(FULL_DIMS.items())), dm["B2"])
    if key not in _CACHE:
        _CACHE[key] = build_program(dm)
    nc = _CACHE[key]
    import os
    trace = os.environ.get("DMPNN_TRACE", "") == "1"
    res = run_bass_kernel_spmd(nc, in_maps, core_ids=list(range(dm["ncores"])),
                               trace=trace, trace_cores=[0] if trace else None)
    global LAST_RESULT
    LAST_RESULT = res

    EC = dm["EC"]
    B2 = dm["B2"]
    out = np.empty((dm["E"], dm["D"]), np.float32)
    for c in range(dm["ncores"]):
        yc = res.results[c]["y"]
        meta = metas[c]
        oc = out[c * EC:(c + 1) * EC]
        nd = meta["ndummy"]
        for i, (base, wdt) in enumerate(zip(meta["bases"], meta["widths"])):
            b = nd + i
            oc[base:base + wdt] = yc[b * 128:b * 128 + wdt]
        oc[EC - 128:] = yc[B2 * 128:(B2 + 1) * 128]
    return out



# revision 15
# speedup vs baseline: 1.3352x; 1.3352x over previous
"""D-MPNN layer on 8 TRN2 NeuronCores (Bass/Tile, SPMD) — v2 (fp8 DoubleRow).

out = (1-z)*s + z*m with
  mess_ki = mess[nei_idx]                       [M, D]
  s_ij    = segment_sum(mess_ki, src_idx, E)    [E, D]
  z_ij    = sigmoid([h_ij | s_ij] @ Wz + bz)    [E, D]
  r_ki    = sigmoid([h_ki | mess_ki] @ Wr + br) [M, D]
  r_ij    = segment_sum(r_ki*mess_ki, src, E)   [E, D]
  m_ij    = tanh(h_ij @ W + bw + r_ij @ U)      [E, D]

Sharding: edges E split into 8 contiguous chunks (EC=E/8); each M-row is
routed on host to the core owning its src edge, so segment sums are
core-local (no collectives).  Rows (sorted by src) are greedily packed into
variable-width dst blocks (window <= 128 dst edges, <= 384 rows, padded to
384); a final 4-tile block covers the core's last 128 dst edges.  One static
program for all cores (block count padded to a common B2).

Device pipeline per group (2 blocks, KG=6 row tiles):
  r   = sigmoid(X @ Wr) via 2 fp8-e4m3 DoubleRow matmuls per tile
        (X^T = [h_ki|mess]^T pre-interleaved [Ki,2,rows] on host)
  oh  = onehot(srcrel) built on device (is_equal vs iota)
  agg : onehot-stationary matmuls accumulate row-major s and r_ij into
        group-wide PSUM banks [s_b0|s_b1], [r_b0|r_b1]
  s^T/r^T via identity matmuls; copied to SBUF as fp8 DoubleRow planes
  z/m : 4 fp8 DoubleRow matmuls per block into [z_b0|z_b1], [m_b0|m_b1]
  combine (1-z)s+zm batched 512 wide; one output DMA per group
"""

import numpy as np
import ml_dtypes

BF16 = ml_dtypes.bfloat16
F8 = ml_dtypes.float8_e4m3

E = 262144
M = 786432
F_NB = 192
D = 256
NCORES = 8

FULL_DIMS = dict(E=E, M=M, F=F_NB, D=D, ncores=NCORES, BLK=128, C=384,
                 CT=512, KG=6)

# fp8 toggles (host layout + device program must agree)
FP8_R = True    # r-phase matmuls in fp8 DoubleRow
FP8_ZM = True   # z/m matmuls in fp8 DoubleRow


def _dims(d, B2):
    o = dict(d)
    o["B2"] = B2
    o["EC"] = o["E"] // o["ncores"]
    o["TPB"] = o["C"] // 128              # 3 row tiles per normal block
    o["TPT"] = o["CT"] // 128             # 4 row tiles in the tail block
    assert o["KG"] == 2 * o["TPB"]
    assert B2 % 2 == 0
    o["G"] = B2 // 2
    o["T"] = o["TPB"] * B2 + o["TPT"]     # total row tiles per core
    return o


def _greedy_blocks(csum, EC, C):
    bases = []
    i = 0
    while i < EC - 128:
        base = i
        hi = min(base + 128, EC - 128)
        j = int(np.searchsorted(csum, csum[base] + C, side="right")) - 1
        j = max(base + 1, min(j, hi))
        bases.append(base)
        i = j
    return bases


def _f8(a):
    return np.clip(a, -240.0, 240.0).astype(F8)


def _interleave_w(w8, lo, hi, ki, ncol):
    """Weight rows [lo:hi) -> DoubleRow [ki, 2, ncol] -> [128, 2*ncol],
    pairing (lo+k, lo+ki+k)."""
    assert hi - lo == 2 * ki
    a = w8[lo:hi].reshape(2, ki, ncol).transpose(1, 0, 2).reshape(ki, 2 * ncol)
    out = np.zeros((128, 2 * ncol), F8)
    out[:ki] = a
    return out


def host_prep(inputs, dims=FULL_DIMS):
    dm0 = dict(dims)
    EC = dm0["E"] // dm0["ncores"]
    C, CT, KG = dm0["C"], dm0["CT"], dm0["KG"]
    F, Dd = dm0["F"], dm0["D"]
    ncores = dm0["ncores"]
    TPB = C // 128

    src = np.asarray(inputs["src_idx"]).astype(np.int64).ravel()
    nei = np.asarray(inputs["nei_idx"]).astype(np.int64).ravel()
    h_ij = np.asarray(inputs["h_ij"])
    h_ki = np.asarray(inputs["h_ki"])
    mess = np.asarray(inputs["mess"])

    order = np.argsort(src, kind="stable")
    src_s = src[order]
    cnt = np.bincount(src_s, minlength=dm0["E"])

    core_blocks = []
    for c in range(ncores):
        csum = np.concatenate(
            [[0], np.cumsum(cnt[c * EC:(c + 1) * EC])]
        )
        bases = _greedy_blocks(csum, EC, C)
        tail_rows = csum[EC] - csum[EC - 128]
        if tail_rows > CT:
            raise OverflowError(f"tail rows {tail_rows} > CT={CT}")
        core_blocks.append((bases, csum))
    nreal = [len(b[0]) for b in core_blocks]
    B2 = max(nreal)
    B2 += B2 % 2
    dm = _dims(dm0, B2)
    G, T = dm["G"], dm["T"]
    TPT = dm["TPT"]

    mess_bf = mess.astype(BF16)
    h_ki_s = h_ki[order]
    nei_s = nei[order]
    mess_g_all = mess_bf[nei_s]            # [M, D] gathered, src-sorted

    # ---- weights ----
    wr = np.asarray(inputs["Wr_w"]).astype(np.float32)   # [448, 256]
    wz = np.asarray(inputs["Wz_w"]).astype(np.float32)   # [448, 256]
    u = np.asarray(inputs["U_w"]).astype(np.float32)     # [256, 256]
    w = np.asarray(inputs["W_w"]).astype(np.float32)     # [192, 256]
    wr8 = _f8(wr)
    wmap = dict(
        wr_dr1=_interleave_w(wr8, 0, 256, 128, Dd),
        wr_dr2=_interleave_w(wr8, 256, 448, 96, Dd),
        wz=np.ascontiguousarray(wz.astype(BF16)),
        u=np.ascontiguousarray(u.astype(BF16)),
        w=np.ascontiguousarray(w.astype(BF16)),
    )

    row_lo = np.searchsorted(src_s, np.arange(ncores) * EC)
    row_hi = np.searchsorted(src_s, (np.arange(ncores) + 1) * EC)

    in_maps = []
    metas = []
    for c in range(ncores):
        bases, csum = core_blocks[c]
        nb = len(bases)
        ndummy = B2 - nb
        MPC = B2 * C + CT
        rlo = row_lo[c]
        nrow_core = row_hi[c] - rlo

        bases_arr = np.asarray(bases, dtype=np.int64)
        nexts = np.concatenate([bases_arr[1:], [EC - 128]])
        widths = nexts - bases_arr
        rs = csum[bases_arr]               # first row of each block
        tail_start = csum[EC - 128]

        rowblk = np.zeros(nrow_core, np.int64)
        rowblk[rs[1:][rs[1:] < nrow_core]] += 1
        rowblk = np.cumsum(rowblk)
        blk_of_row = np.minimum(rowblk, nb - 1)
        ridx = np.arange(nrow_core)
        is_tail = ridx >= tail_start
        pos_in_blk = ridx - rs[blk_of_row]
        slot_of_row = np.where(
            is_tail,
            B2 * C + (ridx - tail_start),
            (ndummy + blk_of_row) * C + pos_in_blk,
        )
        base_of_row = np.where(is_tail, EC - 128, bases_arr[blk_of_row])
        srcrel_pad = np.full(MPC, 999.0, np.float32)
        srcrel_pad[slot_of_row] = (
            src_s[rlo:row_hi[c]] - c * EC - base_of_row
        ).astype(np.float32)

        # padded per-row data
        x_pad = np.zeros((MPC, F + Dd), np.float32)
        x_pad[slot_of_row, :F] = h_ki_s[rlo:row_hi[c]]
        x_pad[slot_of_row, F:] = mess_g_all[rlo:row_hi[c]].astype(np.float32)
        x8 = _f8(x_pad)                    # [MPC, 448] fp8
        mg_pad = np.zeros((MPC, Dd), BF16)
        mg_pad[slot_of_row] = mess_g_all[rlo:row_hi[c]]

        # h_ij per block window [B2+1, 128, F]
        hijc = h_ij[c * EC:(c + 1) * EC].astype(BF16)
        gather_rows = bases_arr[:, None] + np.arange(128)[None, :]
        hij_all = np.zeros((B2 + 1, 128, F), BF16)
        hij_all[ndummy:B2] = hijc[gather_rows]
        hij_all[B2] = hijc[EC - 128:]
        hijt = hij_all.transpose(0, 2, 1)  # [B2+1, 192, 128] bf16

        # ---- per-tile fp8 X^T DoubleRow sections ----
        # xdr1: [128,2,128] pairing (f, f+128), f in 0..128  -> 256 cols
        # xdr2: [96,2,128] pairing (256+f, 352+f) -> 256 cols (pad to 128p)
        xt = x8[:T * 128].reshape(T, 128, F + Dd)
        xdr1 = (xt[:, :, 0:256].transpose(0, 2, 1)   # [T, 256f, 128r]
                .reshape(T, 2, 128, 128).transpose(0, 2, 1, 3)
                .reshape(T, 128, 256))
        x2 = (xt[:, :, 256:448].transpose(0, 2, 1)   # [T, 192f, 128r]
              .reshape(T, 2, 96, 128).transpose(0, 2, 1, 3)
              .reshape(T, 96, 256))
        xdr2 = np.zeros((T, 128, 256), F8)
        xdr2[:, :96] = x2

        # mess row-major bf16, tile-major: [T, 128, 256] -> per-group concat
        mg_t = mg_pad.reshape(T, 128, Dd)

        # ---- group blobs ----
        NT = dm["TPB"] * B2                # = KG * G tiles in normal blocks
        xdr1_g = xdr1[:NT].reshape(G, KG, 128, 256)
        xdr2_g = xdr2[:NT].reshape(G, KG, 128, 256)
        blob8 = np.concatenate([
            xdr1_g.transpose(0, 2, 1, 3).reshape(G, 128, KG * 256),
            xdr2_g.transpose(0, 2, 1, 3).reshape(G, 128, KG * 256),
        ], axis=2)
        # bf16 blob: mg_l | hija (2*128) | hijb (128, partition-packed)
        hija_g = (hijt[:B2, :128, :].reshape(G, 2, 128, 128)
                  .transpose(0, 2, 1, 3).reshape(G, 128, 256))
        hijb_g = hijt[:B2, 128:F, :].reshape(G, 128, 128)
        blobb = np.concatenate([
            mg_t[:NT].reshape(G, KG, 128, Dd)
            .transpose(0, 2, 1, 3).reshape(G, 128, KG * Dd),
            hija_g, hijb_g,
        ], axis=2)

        # ---- tail sections (TPT=4 tiles, 1 block) ----
        t0 = NT
        tail8 = np.concatenate([
            xdr1[t0:].transpose(1, 0, 2).reshape(128, TPT * 256),
            xdr2[t0:].transpose(1, 0, 2).reshape(128, TPT * 256),
        ], axis=1)
        hijbt = np.zeros((128, 128), BF16)
        hijbt[:64] = hijt[B2, 128:F, :]
        tailb = np.concatenate([
            mg_t[t0:].transpose(1, 0, 2).reshape(128, TPT * Dd),
            hijt[B2, :128, :], hijbt,
        ], axis=1)

        src_all = np.ascontiguousarray(srcrel_pad.reshape(T, 128).T)

        im = dict(srcrel=src_all,
                  blob8=np.ascontiguousarray(blob8),
                  blobb=np.ascontiguousarray(blobb),
                  tail8=np.ascontiguousarray(tail8),
                  tailb=np.ascontiguousarray(tailb))
        im.update(wmap)
        in_maps.append(im)
        metas.append(dict(bases=bases_arr, widths=widths, ndummy=ndummy))
    return in_maps, metas, dm


def build_program(dm):
    import concourse.tile as tile
    from concourse import bacc, mybir

    EC, KG, T, G, B2 = dm["EC"], dm["KG"], dm["T"], dm["G"], dm["B2"]
    TPB, TPT, F, Dd = dm["TPB"], dm["TPT"], dm["F"], dm["D"]
    f32 = mybir.dt.float32
    bf16 = mybir.dt.bfloat16
    fp8 = mybir.dt.float8e4
    i32 = mybir.dt.int32
    AF = mybir.ActivationFunctionType
    ALU = mybir.AluOpType
    DR = mybir.MatmulPerfMode.DoubleRow

    nc = bacc.Bacc("TRN2", target_bir_lowering=False, debug=False,
                   num_devices=dm["ncores"])

    NF8 = KG * 256 * 2
    NBF = KG * Dd + 256 + 128
    NT8 = TPT * 256 * 2
    NTB = TPT * Dd + 128 + 128

    srcrel_d = nc.dram_tensor("srcrel", [128, T], f32, kind="ExternalInput")
    blob8_d = nc.dram_tensor("blob8", [G, 128, NF8], fp8, kind="ExternalInput")
    blobb_d = nc.dram_tensor("blobb", [G, 128, NBF], bf16,
                             kind="ExternalInput")
    tail8_d = nc.dram_tensor("tail8", [128, NT8], fp8, kind="ExternalInput")
    tailb_d = nc.dram_tensor("tailb", [128, NTB], bf16, kind="ExternalInput")
    wnames = ["wr_dr1", "wr_dr2"]
    wd = {n: nc.dram_tensor(n, [128, 512], fp8, kind="ExternalInput")
          for n in wnames}
    wz_d = nc.dram_tensor("wz", [448, Dd], bf16, kind="ExternalInput")
    u_d = nc.dram_tensor("u", [Dd, Dd], bf16, kind="ExternalInput")
    w_d = nc.dram_tensor("w", [F, Dd], bf16, kind="ExternalInput")
    y_d = nc.dram_tensor("y", [(B2 + 1) * 128, Dd], f32,
                         kind="ExternalOutput")

    def dr3(ap, ko=2):
        return ap.rearrange("p (ko n) -> p ko n", ko=ko)

    with tile.TileContext(nc) as tc:
        with (
            tc.tile_pool(name="const", bufs=1) as const,
            tc.tile_pool(name="gat", bufs=3) as gat,
            tc.tile_pool(name="mid", bufs=3) as mid,
            tc.tile_pool(name="fin", bufs=3) as fin,
            tc.tile_pool(name="psPR", bufs=2, space="PSUM") as psPR,
            tc.tile_pool(name="psS", bufs=2, space="PSUM") as psS,
            tc.tile_pool(name="psR", bufs=1, space="PSUM") as psR,
            tc.tile_pool(name="psT", bufs=1, space="PSUM") as psT,
            tc.tile_pool(name="psZ", bufs=1, space="PSUM") as psZ,
            tc.tile_pool(name="psM", bufs=1, space="PSUM") as psM,
        ):
            iota_i = const.tile([128, 128], i32)
            nc.gpsimd.iota(iota_i[:], pattern=[[1, 128]], base=0,
                           channel_multiplier=0)
            iota_f = const.tile([128, 128], f32)
            nc.vector.tensor_copy(iota_f[:], iota_i[:])
            iotap_i = const.tile([128, 128], i32)
            nc.gpsimd.iota(iotap_i[:], pattern=[[0, 128]], base=0,
                           channel_multiplier=1)
            ident = const.tile([128, 128], bf16)
            nc.vector.tensor_tensor(
                out=ident[:], in0=iotap_i[:], in1=iota_i[:], op=ALU.is_equal,
            )

            wt = {}
            for n in wnames:
                t = const.tile([128, 512], fp8, tag=n)
                nc.sync.dma_start(out=t[:], in_=wd[n][:, :])
                wt[n] = t

            def load_w(dram, r0, k, nm):
                t = const.tile([k, Dd], bf16, tag=nm)
                nc.sync.dma_start(out=t[:], in_=dram[r0:r0 + k, :])
                return t

            def load_w2(dram, r0, nm):
                t = const.tile([128, Dd], bf16, tag=nm)
                nc.sync.dma_start(out=t[0:64, :], in_=dram[r0:r0 + 64, :])
                nc.sync.dma_start(out=t[64:128, :], in_=dram[r0:r0 + 64, :])
                return t

            wz0 = load_w(wz_d, 0, 128, "wz0")
            wz1d = load_w2(wz_d, 128, "wz1d")
            wz2 = load_w(wz_d, 192, 128, "wz2")
            wz3 = load_w(wz_d, 320, 128, "wz3")
            w0 = load_w(w_d, 0, 128, "w0")
            w1d = load_w2(w_d, 128, "w1d")
            u0 = load_w(u_d, 0, 128, "u0")
            u1 = load_w(u_d, 128, 128, "u1")

            src_all = const.tile([128, T], f32)
            nc.sync.dma_start(out=src_all[:], in_=srcrel_d[:, :])

            def do_group(ntile, nblk, t0, b8, bb_, is_tail):
                """ntile row tiles, nblk dst blocks (<=2).
                b8: fp8 sections tile  bb_: bf16 sections tile."""
                x1o = 0
                x2o = ntile * 256
                hija_o = ntile * Dd
                hijb_o = hija_o + nblk * 128
                TB = ntile // nblk         # tiles per block

                # ---- onehot (gpsimd) ----
                oh_g = mid.tile([128, KG, 128], bf16, tag="oh")
                nc.vector.tensor_tensor(
                    out=oh_g[:, :ntile, :],
                    in0=src_all[:, t0:t0 + ntile, None].broadcast_to(
                        [128, ntile, 128]),
                    in1=iota_f[:, None, :].broadcast_to([128, ntile, 128]),
                    op=ALU.is_equal,
                )

                # ---- r phase: fp8 DoubleRow ----
                r_g = mid.tile([128, KG * Dd], bf16, tag="rg")
                for jj in range(0, ntile, 2):
                    npair = min(2, ntile - jj)
                    pr2 = psPR.tile([128, 512], f32, tag="pr2")
                    for q in range(npair):
                        j = jj + q
                        x1 = dr3(b8[:, x1o + j * 256:x1o + (j + 1) * 256])
                        x2 = dr3(b8[0:96, x2o + j * 256:x2o + (j + 1) * 256])
                        po = pr2[:, q * 256:(q + 1) * 256]
                        nc.tensor.matmul(out=po, lhsT=x1,
                                         rhs=dr3(wt["wr_dr1"][:]),
                                         start=True, stop=False, perf_mode=DR)
                        nc.tensor.matmul(out=po, lhsT=x2,
                                         rhs=dr3(wt["wr_dr2"][0:96, :]),
                                         start=False, stop=True, perf_mode=DR)
                    nc.scalar.activation(
                        r_g[:, jj * Dd:(jj + npair) * Dd],
                        pr2[:, :npair * 256], AF.Sigmoid)

                rm_g = mid.tile([128, KG * Dd], bf16, tag="rm")
                nc.vector.tensor_tensor(out=rm_g[:, :ntile * Dd],
                                        in0=r_g[:, :ntile * Dd],
                                        in1=bb_[:, :ntile * Dd],
                                        op=ALU.mult)

                # ---- aggregation into group-wide banks ----
                ps_s = psS.tile([128, 512], f32, tag="ps_s")
                ps_r = psR.tile([128, 512], f32, tag="ps_r")
                for bbk in range(nblk):
                    for tj in range(TB):
                        j = bbk * TB + tj
                        oh = oh_g[:, j, :]
                        st, sp = tj == 0, tj == TB - 1
                        sl = slice(bbk * 256, bbk * 256 + 256)
                        nc.tensor.matmul(out=ps_s[:, sl], lhsT=oh,
                                         rhs=bb_[:, j * Dd:(j + 1) * Dd],
                                         start=st, stop=sp)
                        nc.tensor.matmul(out=ps_r[:, sl], lhsT=oh,
                                         rhs=rm_g[:, j * Dd:(j + 1) * Dd],
                                         start=st, stop=sp)

                # ---- copies + transposes + z/m ----
                W = nblk * 256
                c1 = fin.tile([128, 1024], bf16, tag="c1")
                nc.vector.tensor_copy(c1[:, 0:W], ps_s[:, 0:W])
                nc.vector.tensor_copy(c1[:, W:2 * W], ps_r[:, 0:W])

                pz = psZ.tile([128, 512], f32, tag="pz")
                pm = psM.tile([128, 512], f32, tag="pm")
                for bbk in range(nblk):
                    pst = psT.tile([128, 512], f32, tag="pst")
                    for k in range(2):
                        nc.tensor.matmul(
                            out=pst[:, k * 128:(k + 1) * 128],
                            lhsT=c1[:, bbk * 256 + k * 128:
                                    bbk * 256 + (k + 1) * 128],
                            rhs=ident[:], start=True, stop=True)
                        nc.tensor.matmul(
                            out=pst[:, 256 + k * 128:256 + (k + 1) * 128],
                            lhsT=c1[:, W + bbk * 256 + k * 128:
                                    W + bbk * 256 + (k + 1) * 128],
                            rhs=ident[:], start=True, stop=True)
                    c2 = fin.tile([128, 512], bf16, tag="c2")
                    nc.scalar.activation(c2[:], pst[:], AF.Copy)

                    hija = bb_[:, hija_o + bbk * 128:hija_o + (bbk + 1) * 128]
                    hijb = bb_[bbk * 64:(bbk + 1) * 64,
                               hijb_o:hijb_o + 128]
                    zsl = slice(bbk * 256, bbk * 256 + 256)
                    nc.tensor.matmul(out=pz[:, zsl], lhsT=hija, rhs=wz0[:],
                                     start=True, stop=False)
                    nc.tensor.matmul(out=pm[:, zsl], lhsT=hija, rhs=w0[:],
                                     start=True, stop=False)
                    nc.tensor.matmul(out=pz[:, zsl], lhsT=hijb,
                                     rhs=wz1d[bbk * 64:(bbk + 1) * 64, :],
                                     start=False, stop=False)
                    nc.tensor.matmul(out=pm[:, zsl], lhsT=hijb,
                                     rhs=w1d[bbk * 64:(bbk + 1) * 64, :],
                                     start=False, stop=False)
                    nc.tensor.matmul(out=pz[:, zsl], lhsT=c2[:, 0:128],
                                     rhs=wz2[:], start=False, stop=False)
                    nc.tensor.matmul(out=pz[:, zsl], lhsT=c2[:, 128:256],
                                     rhs=wz3[:], start=False, stop=True)
                    nc.tensor.matmul(out=pm[:, zsl], lhsT=c2[:, 256:384],
                                     rhs=u0[:], start=False, stop=False)
                    nc.tensor.matmul(out=pm[:, zsl], lhsT=c2[:, 384:512],
                                     rhs=u1[:], start=False, stop=True)

                # ---- activations + combine (batched) ----
                z_sb = fin.tile([128, 512], f32, tag="z")
                nc.scalar.activation(z_sb[:, :W], pz[:, :W], AF.Sigmoid)
                m_sb = fin.tile([128, 512], f32, tag="m")
                nc.scalar.activation(m_sb[:, :W], pm[:, :W], AF.Tanh)
                t1 = fin.tile([128, 512], f32, tag="t1")
                nc.vector.tensor_tensor(out=t1[:, :W], in0=m_sb[:, :W],
                                        in1=ps_s[:, :W], op=ALU.subtract)
                nc.vector.tensor_tensor(out=t1[:, :W], in0=t1[:, :W],
                                        in1=z_sb[:, :W], op=ALU.mult)
                o_sb = fin.tile([128, 512], f32, tag="o")
                nc.vector.tensor_tensor(out=o_sb[:, :W], in0=t1[:, :W],
                                        in1=ps_s[:, :W], op=ALU.add)
                return o_sb

            for g in range(G):
                b8 = gat.tile([128, NF8], fp8, tag="b8")
                nc.sync.dma_start(out=b8[:], in_=blob8_d[g])
                bb_ = gat.tile([128, NBF], bf16, tag="bb")
                nc.sync.dma_start(out=bb_[:], in_=blobb_d[g])
                o_sb = do_group(KG, 2, g * KG, b8, bb_, False)
                yv = y_d[2 * g * 128:(2 * g + 2) * 128, :].rearrange(
                    "(bb p) d -> p bb d", bb=2)
                nc.sync.dma_start(
                    out=yv,
                    in_=o_sb[:].rearrange("p (bb d) -> p bb d", bb=2))

            # tail block
            t8 = gat.tile([128, NT8], fp8, tag="t8")
            nc.sync.dma_start(out=t8[:], in_=tail8_d[:, :])
            tb_ = gat.tile([128, NTB], bf16, tag="tb")
            nc.sync.dma_start(out=tb_[:], in_=tailb_d[:, :])
            o_sb = do_group(TPT, 1, B2 * TPB, t8, tb_, True)
            nc.sync.dma_start(out=y_d[B2 * 128:(B2 + 1) * 128, :],
                              in_=o_sb[:, 0:256])

    nc.compile()
    return nc


_CACHE = {}
LAST_RESULT = None


def kernel(**inputs):
    from concourse.bass_utils import run_bass_kernel_spmd

    for b in ("Wz_b", "Wr_b", "W_b"):
        assert not np.any(np.asarray(inputs[b])), f"nonzero bias {b} unsupported"

    in_maps, metas, dm = host_prep(inputs, FULL_DIMS)
    key = (tuple(sorted(FULL_DIMS.items())), dm["B2"])
    if key not in _CACHE:
        _CACHE[key] = build_program(dm)
    nc = _CACHE[key]
    import os
    trace = os.environ.get("DMPNN_TRACE", "") == "1"
    res = run_bass_kernel_spmd(nc, in_maps, core_ids=list(range(dm["ncores"])),
                               trace=trace, trace_cores=[0] if trace else None)
    global LAST_RESULT
    LAST_RESULT = res

    EC = dm["EC"]
    B2 = dm["B2"]
    out = np.empty((dm["E"], dm["D"]), np.float32)
    for c in range(dm["ncores"]):
        yc = res.results[c]["y"]
        meta = metas[c]
        oc = out[c * EC:(c + 1) * EC]
        nd = meta["ndummy"]
        for i, (base, wdt) in enumerate(zip(meta["bases"], meta["widths"])):
            b = nd + i
            oc[base:base + wdt] = yc[b * 128:b * 128 + wdt]
        oc[EC - 128:] = yc[B2 * 128:(B2 + 1) * 128]
    return out


# revision 23
# speedup vs baseline: 1.4361x; 1.0756x over previous
"""D-MPNN layer on 8 TRN2 NeuronCores (Bass/Tile, SPMD) — v2 (fp8 DoubleRow).

out = (1-z)*s + z*m with
  mess_ki = mess[nei_idx]                       [M, D]
  s_ij    = segment_sum(mess_ki, src_idx, E)    [E, D]
  z_ij    = sigmoid([h_ij | s_ij] @ Wz + bz)    [E, D]
  r_ki    = sigmoid([h_ki | mess_ki] @ Wr + br) [M, D]
  r_ij    = segment_sum(r_ki*mess_ki, src, E)   [E, D]
  m_ij    = tanh(h_ij @ W + bw + r_ij @ U)      [E, D]

Sharding: edges E split into 8 contiguous chunks (EC=E/8); each M-row is
routed on host to the core owning its src edge, so segment sums are
core-local (no collectives).  Rows (sorted by src) are greedily packed into
variable-width dst blocks (window <= 128 dst edges, <= 384 rows, padded to
384); a final 4-tile block covers the core's last 128 dst edges.  One static
program for all cores (block count padded to a common B2).

Device pipeline per group (2 blocks, KG=6 row tiles):
  r   = sigmoid(X @ Wr) via 2 fp8-e4m3 DoubleRow matmuls per tile
        (X^T = [h_ki|mess]^T pre-interleaved [Ki,2,rows] on host)
  oh  = onehot(srcrel) built on device (is_equal vs iota)
  agg : onehot-stationary matmuls accumulate row-major s and r_ij into
        group-wide PSUM banks [s_b0|s_b1], [r_b0|r_b1]
  s^T/r^T via identity matmuls; copied to SBUF as fp8 DoubleRow planes
  z/m : 4 fp8 DoubleRow matmuls per block into [z_b0|z_b1], [m_b0|m_b1]
  combine (1-z)s+zm batched 512 wide; one output DMA per group
"""

import numpy as np
import ml_dtypes

BF16 = ml_dtypes.bfloat16
F8 = ml_dtypes.float8_e4m3

E = 262144
M = 786432
F_NB = 192
D = 256
NCORES = 8

FULL_DIMS = dict(E=E, M=M, F=F_NB, D=D, ncores=NCORES, BLK=128, C=384,
                 CT=512, KG=6)

# fp8 toggles (host layout + device program must agree)
FP8_R = True    # r-phase matmuls in fp8 DoubleRow
FP8_ZM = True   # z/m matmuls in fp8 DoubleRow


def _dims(d, B2):
    o = dict(d)
    o["B2"] = B2
    o["EC"] = o["E"] // o["ncores"]
    o["TPB"] = o["C"] // 128              # 3 row tiles per normal block
    o["TPT"] = o["CT"] // 128             # 4 row tiles in the tail block
    assert o["KG"] == 2 * o["TPB"]
    assert B2 % 2 == 0
    o["G"] = B2 // 2
    o["T"] = o["TPB"] * B2 + o["TPT"]     # total row tiles per core
    return o


def _greedy_blocks(csum, EC, C):
    bases = []
    i = 0
    while i < EC - 128:
        base = i
        hi = min(base + 128, EC - 128)
        j = int(np.searchsorted(csum, csum[base] + C, side="right")) - 1
        j = max(base + 1, min(j, hi))
        bases.append(base)
        i = j
    return bases


def _f8(a):
    return np.clip(a, -240.0, 240.0).astype(F8)


def _interleave_w(w8, lo, hi, ki, ncol):
    """Weight rows [lo:hi) -> DoubleRow [ki, 2, ncol] -> [128, 2*ncol],
    pairing (lo+k, lo+ki+k)."""
    assert hi - lo == 2 * ki
    a = w8[lo:hi].reshape(2, ki, ncol).transpose(1, 0, 2).reshape(ki, 2 * ncol)
    out = np.zeros((128, 2 * ncol), F8)
    out[:ki] = a
    return out


def host_prep(inputs, dims=FULL_DIMS):
    dm0 = dict(dims)
    EC = dm0["E"] // dm0["ncores"]
    C, CT, KG = dm0["C"], dm0["CT"], dm0["KG"]
    F, Dd = dm0["F"], dm0["D"]
    ncores = dm0["ncores"]
    TPB = C // 128

    src = np.asarray(inputs["src_idx"]).astype(np.int64).ravel()
    nei = np.asarray(inputs["nei_idx"]).astype(np.int64).ravel()
    h_ij = np.asarray(inputs["h_ij"])
    h_ki = np.asarray(inputs["h_ki"])
    mess = np.asarray(inputs["mess"])

    order = np.argsort(src, kind="stable")
    src_s = src[order]
    cnt = np.bincount(src_s, minlength=dm0["E"])

    core_blocks = []
    for c in range(ncores):
        csum = np.concatenate(
            [[0], np.cumsum(cnt[c * EC:(c + 1) * EC])]
        )
        bases = _greedy_blocks(csum, EC, C)
        tail_rows = csum[EC] - csum[EC - 128]
        if tail_rows > CT:
            raise OverflowError(f"tail rows {tail_rows} > CT={CT}")
        core_blocks.append((bases, csum))
    nreal = [len(b[0]) for b in core_blocks]
    B2 = max(nreal)
    B2 += B2 % 2
    dm = _dims(dm0, B2)
    G, T = dm["G"], dm["T"]
    TPT = dm["TPT"]

    mess_bf = mess.astype(BF16)
    h_ki_s = h_ki[order]
    nei_s = nei[order]
    mess_g_all = mess_bf[nei_s]            # [M, D] gathered, src-sorted

    # ---- weights ----
    wr = np.asarray(inputs["Wr_w"]).astype(np.float32)   # [448, 256]
    wz = np.asarray(inputs["Wz_w"]).astype(np.float32)   # [448, 256]
    u = np.asarray(inputs["U_w"]).astype(np.float32)     # [256, 256]
    w = np.asarray(inputs["W_w"]).astype(np.float32)     # [192, 256]
    wr8 = _f8(wr)
    wmap = dict(
        wr_dr1=_interleave_w(wr8, 0, 256, 128, Dd),
        wr_dr2=_interleave_w(wr8, 256, 448, 96, Dd),
        wz=np.ascontiguousarray(wz.astype(BF16)),
        u=np.ascontiguousarray(u.astype(BF16)),
        w=np.ascontiguousarray(w.astype(BF16)),
    )

    row_lo = np.searchsorted(src_s, np.arange(ncores) * EC)
    row_hi = np.searchsorted(src_s, (np.arange(ncores) + 1) * EC)

    in_maps = []
    metas = []
    for c in range(ncores):
        bases, csum = core_blocks[c]
        nb = len(bases)
        ndummy = B2 - nb
        MPC = B2 * C + CT
        rlo = row_lo[c]
        nrow_core = row_hi[c] - rlo

        bases_arr = np.asarray(bases, dtype=np.int64)
        nexts = np.concatenate([bases_arr[1:], [EC - 128]])
        widths = nexts - bases_arr
        rs = csum[bases_arr]               # first row of each block
        tail_start = csum[EC - 128]

        rowblk = np.zeros(nrow_core, np.int64)
        rowblk[rs[1:][rs[1:] < nrow_core]] += 1
        rowblk = np.cumsum(rowblk)
        blk_of_row = np.minimum(rowblk, nb - 1)
        ridx = np.arange(nrow_core)
        is_tail = ridx >= tail_start
        pos_in_blk = ridx - rs[blk_of_row]
        slot_of_row = np.where(
            is_tail,
            B2 * C + (ridx - tail_start),
            (ndummy + blk_of_row) * C + pos_in_blk,
        )
        base_of_row = np.where(is_tail, EC - 128, bases_arr[blk_of_row])
        srcrel_pad = np.full(MPC, 999.0, np.float32)
        srcrel_pad[slot_of_row] = (
            src_s[rlo:row_hi[c]] - c * EC - base_of_row
        ).astype(np.float32)

        # padded per-row data
        x_pad = np.zeros((MPC, F + Dd), np.float32)
        x_pad[slot_of_row, :F] = h_ki_s[rlo:row_hi[c]]
        x_pad[slot_of_row, F:] = mess_g_all[rlo:row_hi[c]].astype(np.float32)
        x8 = _f8(x_pad)                    # [MPC, 448] fp8
        mg_pad = np.zeros((MPC, Dd), BF16)
        mg_pad[slot_of_row] = mess_g_all[rlo:row_hi[c]]

        # h_ij per block window [B2+1, 128, F]
        hijc = h_ij[c * EC:(c + 1) * EC].astype(BF16)
        gather_rows = bases_arr[:, None] + np.arange(128)[None, :]
        hij_all = np.zeros((B2 + 1, 128, F), BF16)
        hij_all[ndummy:B2] = hijc[gather_rows]
        hij_all[B2] = hijc[EC - 128:]
        hijt = hij_all.transpose(0, 2, 1)  # [B2+1, 192, 128] bf16

        # ---- per-tile fp8 X^T DoubleRow sections ----
        # xdr1: [128,2,128] pairing (f, f+128), f in 0..128  -> 256 cols
        # xdr2: [96,2,128] pairing (256+f, 352+f) -> 256 cols (pad to 128p)
        xt = x8[:T * 128].reshape(T, 128, F + Dd)
        xdr1 = (xt[:, :, 0:256].transpose(0, 2, 1)   # [T, 256f, 128r]
                .reshape(T, 2, 128, 128).transpose(0, 2, 1, 3)
                .reshape(T, 128, 256))
        x2 = (xt[:, :, 256:448].transpose(0, 2, 1)   # [T, 192f, 128r]
              .reshape(T, 2, 96, 128).transpose(0, 2, 1, 3)
              .reshape(T, 96, 256))
        xdr2 = np.zeros((T, 128, 256), F8)
        xdr2[:, :96] = x2

        # mess row-major bf16, tile-major: [T, 128, 256] -> per-group concat
        mg_t = mg_pad.reshape(T, 128, Dd)

        # ---- group blobs ----
        NT = dm["TPB"] * B2                # = KG * G tiles in normal blocks
        xdr1_g = xdr1[:NT].reshape(G, KG, 128, 256)
        xdr2_g = xdr2[:NT].reshape(G, KG, 128, 256)
        blob8 = np.concatenate([
            xdr1_g.transpose(0, 2, 1, 3).reshape(G, 128, KG * 256),
            xdr2_g.transpose(0, 2, 1, 3).reshape(G, 128, KG * 256),
        ], axis=2)
        # bf16 blob: mg_l | hija (2*128) | hijb (128, partition-packed)
        hija_g = (hijt[:B2, :128, :].reshape(G, 2, 128, 128)
                  .transpose(0, 2, 1, 3).reshape(G, 128, 256))
        hijb_g = hijt[:B2, 128:F, :].reshape(G, 128, 128)
        blobb = np.concatenate([
            mg_t[:NT].reshape(G, KG, 128, Dd)
            .transpose(0, 2, 1, 3).reshape(G, 128, KG * Dd),
            hija_g, hijb_g,
        ], axis=2)

        # ---- tail sections (TPT=4 tiles, 1 block) ----
        t0 = NT
        tail8 = np.concatenate([
            xdr1[t0:].transpose(1, 0, 2).reshape(128, TPT * 256),
            xdr2[t0:].transpose(1, 0, 2).reshape(128, TPT * 256),
        ], axis=1)
        hijbt = np.zeros((128, 128), BF16)
        hijbt[:64] = hijt[B2, 128:F, :]
        tailb = np.concatenate([
            mg_t[t0:].transpose(1, 0, 2).reshape(128, TPT * Dd),
            hijt[B2, :128, :], hijbt,
        ], axis=1)

        src_all = np.ascontiguousarray(srcrel_pad.reshape(T, 128).T)

        im = dict(srcrel=src_all,
                  blob8=np.ascontiguousarray(blob8),
                  blobb=np.ascontiguousarray(blobb),
                  tail8=np.ascontiguousarray(tail8),
                  tailb=np.ascontiguousarray(tailb))
        im.update(wmap)
        in_maps.append(im)
        metas.append(dict(bases=bases_arr, widths=widths, ndummy=ndummy))
    return in_maps, metas, dm


def build_program(dm):
    import concourse.tile as tile
    from concourse import bacc, mybir

    EC, KG, T, G, B2 = dm["EC"], dm["KG"], dm["T"], dm["G"], dm["B2"]
    TPB, TPT, F, Dd = dm["TPB"], dm["TPT"], dm["F"], dm["D"]
    f32 = mybir.dt.float32
    bf16 = mybir.dt.bfloat16
    fp8 = mybir.dt.float8e4
    i32 = mybir.dt.int32
    AF = mybir.ActivationFunctionType
    ALU = mybir.AluOpType
    DR = mybir.MatmulPerfMode.DoubleRow

    nc = bacc.Bacc("TRN2", target_bir_lowering=False, debug=False,
                   num_devices=dm["ncores"])

    NF8 = KG * 256 * 2
    NBF = KG * Dd + 256 + 128
    NT8 = TPT * 256 * 2
    NTB = TPT * Dd + 128 + 128

    srcrel_d = nc.dram_tensor("srcrel", [128, T], f32, kind="ExternalInput")
    blob8_d = nc.dram_tensor("blob8", [G, 128, NF8], fp8, kind="ExternalInput")
    blobb_d = nc.dram_tensor("blobb", [G, 128, NBF], bf16,
                             kind="ExternalInput")
    tail8_d = nc.dram_tensor("tail8", [128, NT8], fp8, kind="ExternalInput")
    tailb_d = nc.dram_tensor("tailb", [128, NTB], bf16, kind="ExternalInput")
    wnames = ["wr_dr1", "wr_dr2"]
    wd = {n: nc.dram_tensor(n, [128, 512], fp8, kind="ExternalInput")
          for n in wnames}
    wz_d = nc.dram_tensor("wz", [448, Dd], bf16, kind="ExternalInput")
    u_d = nc.dram_tensor("u", [Dd, Dd], bf16, kind="ExternalInput")
    w_d = nc.dram_tensor("w", [F, Dd], bf16, kind="ExternalInput")
    y_d = nc.dram_tensor("y", [(B2 + 1) * 128, Dd], f32,
                         kind="ExternalOutput")

    def dr3(ap, ko=2):
        return ap.rearrange("p (ko n) -> p ko n", ko=ko)

    with tile.TileContext(nc) as tc:
        with (
            tc.tile_pool(name="const", bufs=1) as const,
            tc.tile_pool(name="gat", bufs=3) as gat,
            tc.tile_pool(name="mid", bufs=3) as mid,
            tc.tile_pool(name="fin", bufs=3) as fin,
            tc.tile_pool(name="psPR", bufs=2, space="PSUM") as psPR,
            tc.tile_pool(name="psSR", bufs=2, space="PSUM") as psSR,
            tc.tile_pool(name="psT", bufs=2, space="PSUM") as psT,
            tc.tile_pool(name="psZ", bufs=1, space="PSUM") as psZ,
            tc.tile_pool(name="psM", bufs=1, space="PSUM") as psM,
        ):
            iota_i = const.tile([128, 128], i32)
            nc.gpsimd.iota(iota_i[:], pattern=[[1, 128]], base=0,
                           channel_multiplier=0)
            iota_f = const.tile([128, 128], f32)
            nc.vector.tensor_copy(iota_f[:], iota_i[:])
            iotap_i = const.tile([128, 128], i32)
            nc.gpsimd.iota(iotap_i[:], pattern=[[0, 128]], base=0,
                           channel_multiplier=1)
            ident = const.tile([128, 128], bf16)
            nc.vector.tensor_tensor(
                out=ident[:], in0=iotap_i[:], in1=iota_i[:], op=ALU.is_equal,
            )

            wt = {}
            for n in wnames:
                t = const.tile([128, 512], fp8, tag=n)
                nc.sync.dma_start(out=t[:], in_=wd[n][:, :])
                wt[n] = t

            def load_w(dram, r0, k, nm):
                t = const.tile([k, Dd], bf16, tag=nm)
                nc.sync.dma_start(out=t[:], in_=dram[r0:r0 + k, :])
                return t

            def load_w2(dram, r0, nm):
                t = const.tile([128, Dd], bf16, tag=nm)
                nc.sync.dma_start(out=t[0:64, :], in_=dram[r0:r0 + 64, :])
                nc.sync.dma_start(out=t[64:128, :], in_=dram[r0:r0 + 64, :])
                return t

            wz0 = load_w(wz_d, 0, 128, "wz0")
            wz1d = load_w2(wz_d, 128, "wz1d")
            wz2 = load_w(wz_d, 192, 128, "wz2")
            wz3 = load_w(wz_d, 320, 128, "wz3")
            w0 = load_w(w_d, 0, 128, "w0")
            w1d = load_w2(w_d, 128, "w1d")
            u0 = load_w(u_d, 0, 128, "u0")
            u1 = load_w(u_d, 128, 128, "u1")

            src_all = const.tile([128, T], f32)
            nc.sync.dma_start(out=src_all[:], in_=srcrel_d[:, :])

            def do_group(ntile, nblk, t0, b8, mrm, bh):
                """ntile row tiles, nblk dst blocks (<=2).
                b8: fp8 sections  mrm: [128,KG,512] [mess|rm]  bh: hij^T."""
                x1o = 0
                x2o = ntile * 256
                TB = ntile // nblk         # tiles per block

                # ---- onehot (gpsimd) ----
                oh_g = mid.tile([128, KG, 128], bf16, tag="oh")
                nc.vector.tensor_tensor(
                    out=oh_g[:, :ntile, :],
                    in0=src_all[:, t0:t0 + ntile, None].broadcast_to(
                        [128, ntile, 128]),
                    in1=iota_f[:, None, :].broadcast_to([128, ntile, 128]),
                    op=ALU.is_equal,
                )

                # ---- r phase: fp8 DoubleRow ----
                r_g = mid.tile([128, KG * Dd], bf16, tag="rg")
                for jj in range(0, ntile, 2):
                    npair = min(2, ntile - jj)
                    pr2 = psPR.tile([128, 512], f32, tag="pr2")
                    for q in range(npair):
                        j = jj + q
                        x1 = dr3(b8[:, x1o + j * 256:x1o + (j + 1) * 256])
                        x2 = dr3(b8[0:96, x2o + j * 256:x2o + (j + 1) * 256])
                        po = pr2[:, q * 256:(q + 1) * 256]
                        nc.tensor.matmul(out=po, lhsT=x1,
                                         rhs=dr3(wt["wr_dr1"][:]),
                                         start=True, stop=False, perf_mode=DR)
                        nc.tensor.matmul(out=po, lhsT=x2,
                                         rhs=dr3(wt["wr_dr2"][0:96, :]),
                                         start=False, stop=True, perf_mode=DR)
                    nc.scalar.activation(
                        r_g[:, jj * Dd:(jj + npair) * Dd],
                        pr2[:, :npair * 256], AF.Sigmoid)

                nc.vector.tensor_tensor(
                    out=mrm[:, :ntile, 256:512],
                    in0=r_g[:, :ntile * Dd].rearrange(
                        "p (j d) -> p j d", j=ntile),
                    in1=mrm[:, :ntile, 0:256],
                    op=ALU.mult)

                # ---- aggregation: one 512-wide MM per tile ----
                W = nblk * 256
                c1 = fin.tile([128, 2, 512], bf16, tag="c1")
                for bbk in range(nblk):
                    ps_sr = psSR.tile([128, 512], f32, tag="ps_sr")
                    for tj in range(TB):
                        j = bbk * TB + tj
                        nc.tensor.matmul(out=ps_sr[:], lhsT=oh_g[:, j, :],
                                         rhs=mrm[:, j, :],
                                         start=tj == 0, stop=tj == TB - 1)
                    nc.vector.tensor_copy(c1[:, bbk, :], ps_sr[:])

                pz = psZ.tile([128, 512], f32, tag="pz")
                pm = psM.tile([128, 512], f32, tag="pm")
                for bbk in range(nblk):
                    pst = psT.tile([128, 512], f32, tag="pst")
                    for k in range(2):
                        nc.tensor.matmul(
                            out=pst[:, k * 128:(k + 1) * 128],
                            lhsT=c1[:, bbk, k * 128:(k + 1) * 128],
                            rhs=ident[:], start=True, stop=True)
                        nc.tensor.matmul(
                            out=pst[:, 256 + k * 128:256 + (k + 1) * 128],
                            lhsT=c1[:, bbk, 256 + k * 128:256 + (k + 1) * 128],
                            rhs=ident[:], start=True, stop=True)
                    c2 = fin.tile([128, 512], bf16, tag="c2")
                    nc.scalar.activation(c2[:], pst[:], AF.Copy)

                    hija = bh[:, bbk * 128:(bbk + 1) * 128]
                    hijb = bh[bbk * 64:(bbk + 1) * 64,
                              nblk * 128:nblk * 128 + 128]
                    zsl = slice(bbk * 256, bbk * 256 + 256)
                    nc.tensor.matmul(out=pz[:, zsl], lhsT=hija, rhs=wz0[:],
                                     start=True, stop=False)
                    nc.tensor.matmul(out=pm[:, zsl], lhsT=hija, rhs=w0[:],
                                     start=True, stop=False)
                    nc.tensor.matmul(out=pz[:, zsl], lhsT=hijb,
                                     rhs=wz1d[bbk * 64:(bbk + 1) * 64, :],
                                     start=False, stop=False)
                    nc.tensor.matmul(out=pm[:, zsl], lhsT=hijb,
                                     rhs=w1d[bbk * 64:(bbk + 1) * 64, :],
                                     start=False, stop=False)
                    nc.tensor.matmul(out=pz[:, zsl], lhsT=c2[:, 0:128],
                                     rhs=wz2[:], start=False, stop=False)
                    nc.tensor.matmul(out=pz[:, zsl], lhsT=c2[:, 128:256],
                                     rhs=wz3[:], start=False, stop=True)
                    nc.tensor.matmul(out=pm[:, zsl], lhsT=c2[:, 256:384],
                                     rhs=u0[:], start=False, stop=False)
                    nc.tensor.matmul(out=pm[:, zsl], lhsT=c2[:, 384:512],
                                     rhs=u1[:], start=False, stop=True)

                # ---- activations + combine (batched) ----
                z_sb = fin.tile([128, 512], f32, tag="z")
                nc.scalar.activation(z_sb[:, :W], pz[:, :W], AF.Sigmoid)
                m_sb = fin.tile([128, 512], f32, tag="m")
                nc.scalar.activation(m_sb[:, :W], pm[:, :W], AF.Tanh)
                s_view = c1[:, 0:nblk, 0:256]
                t1 = fin.tile([128, 512], f32, tag="t1")
                nc.vector.tensor_tensor(out=t1[:, :W], in0=m_sb[:, :W],
                                        in1=s_view, op=ALU.subtract)
                nc.vector.tensor_tensor(out=t1[:, :W], in0=t1[:, :W],
                                        in1=z_sb[:, :W], op=ALU.mult)
                o_sb = fin.tile([128, 512], f32, tag="o")
                nc.vector.tensor_tensor(out=o_sb[:, :W], in0=t1[:, :W],
                                        in1=s_view, op=ALU.add)
                return o_sb

            for g in range(G):
                b8 = gat.tile([128, NF8], fp8, tag="b8")
                nc.sync.dma_start(out=b8[:], in_=blob8_d[g])
                mrm = mid.tile([128, KG, 512], bf16, tag="mrm")
                nc.sync.dma_start(
                    out=mrm[:, :, 0:256],
                    in_=blobb_d[g][:, 0:KG * Dd].rearrange(
                        "p (j d) -> p j d", j=KG))
                bh = gat.tile([128, 384], bf16, tag="bh")
                nc.sync.dma_start(out=bh[:], in_=blobb_d[g][:, KG * Dd:])
                o_sb = do_group(KG, 2, g * KG, b8, mrm, bh)
                yv = y_d[2 * g * 128:(2 * g + 2) * 128, :].rearrange(
                    "(bb p) d -> p bb d", bb=2)
                nc.sync.dma_start(
                    out=yv,
                    in_=o_sb[:].rearrange("p (bb d) -> p bb d", bb=2))

            # tail block
            t8 = gat.tile([128, NT8], fp8, tag="t8")
            nc.sync.dma_start(out=t8[:], in_=tail8_d[:, :])
            mrm = mid.tile([128, KG, 512], bf16, tag="mrm")
            nc.sync.dma_start(
                out=mrm[:, 0:TPT, 0:256],
                in_=tailb_d[:, 0:TPT * Dd].rearrange(
                    "p (j d) -> p j d", j=TPT))
            bh = gat.tile([128, 384], bf16, tag="bh")
            nc.sync.dma_start(out=bh[:, 0:256], in_=tailb_d[:, TPT * Dd:])
            o_sb = do_group(TPT, 1, B2 * TPB, t8, mrm, bh)
            nc.sync.dma_start(out=y_d[B2 * 128:(B2 + 1) * 128, :],
                              in_=o_sb[:, 0:256])

    nc.compile()
    return nc


_CACHE = {}
LAST_RESULT = None


def kernel(**inputs):
    from concourse.bass_utils import run_bass_kernel_spmd

    for b in ("Wz_b", "Wr_b", "W_b"):
        assert not np.any(np.asarray(inputs[b])), f"nonzero bias {b} unsupported"

    in_maps, metas, dm = host_prep(inputs, FULL_DIMS)
    key = (tuple(sorted(FULL_DIMS.items())), dm["B2"])
    if key not in _CACHE:
        _CACHE[key] = build_program(dm)
    nc = _CACHE[key]
    import os
    trace = os.environ.get("DMPNN_TRACE", "") == "1"
    res = run_bass_kernel_spmd(nc, in_maps, core_ids=list(range(dm["ncores"])),
                               trace=trace, trace_cores=[0] if trace else None)
    global LAST_RESULT
    LAST_RESULT = res

    EC = dm["EC"]
    B2 = dm["B2"]
    out = np.empty((dm["E"], dm["D"]), np.float32)
    for c in range(dm["ncores"]):
        yc = res.results[c]["y"]
        meta = metas[c]
        oc = out[c * EC:(c + 1) * EC]
        nd = meta["ndummy"]
        for i, (base, wdt) in enumerate(zip(meta["bases"], meta["widths"])):
            b = nd + i
            oc[base:base + wdt] = yc[b * 128:b * 128 + wdt]
        oc[EC - 128:] = yc[B2 * 128:(B2 + 1) * 128]
    return out
